# revision 32
# baseline (speedup 1.0000x reference)
"""Ragged-sequence multi-head attention (B=16, S=1024, D=512, H=8, DH=64)
for 8 Trainium2 NeuronCores.

Strategy: the 16 sequences (known lengths at call time) are packed into a
set of rectangle types (kb, qb) in 128-token tile units.  Each rect type
is instantiated once per core (SPMD); an instance processes one
(sequence, q-tile-range) piece: attention of qb q-tiles against the
sequence's first kb k-tiles, with per-k-position additive masking of the
exp() activation.  Long sequences split their q-range across cores
(duplicating their K/V projection), short ones ride in small rects.  A
runtime search picks the rect multiset minimizing modeled PE time.

Per-core pipeline per rect (fp16 matmul operands, fp32 accumulation):
  1. xT, xqT arrive pre-transposed (feature-major fp16, host-prepared)
  2. KT = Wk^T x^T, QT = Wq^T xq^T (feature-major), V in [tok, d] layout
  3. per head-pair, per q-chunk, per k-tile:
       scoresT[k, q] = K^T q            (row-packed head pair)
       expT = exp(scale*scoresT + kbias) (ACT engine -> fp16, masked keys -> 0)
       outT[d, q]  += V^T expT          (col-packed head pair)
       denom[., q] += 1^T expT          (col-packed, rows replicated)
  4. outT_norm = outT * reciprocal(denom)
  5. out[tok, d] = outT_norm^T Wo + bo  -> DMA out (host masks padded rows)
"""

import os
from itertools import combinations_with_replacement as cwr

import numpy as np

B, S, D = 16, 1024, 512
H, DH = 8, 64
N_CORES = 8
P = 128
KC = D // P  # 4 contraction chunks of 128

_BUILD_CACHE: dict = {}


def _ntiles(L: int) -> int:
    return max(1, (int(L) + P - 1) // P)


# ---------------------------------------------------------------------------
# Runtime scheduler: pick rect types and assign (seq, q-range) pieces.
# ---------------------------------------------------------------------------

def _attn_unit(q: int) -> float:
    w = min(q, 4) * 128
    return (1050.0 + (512.0 / w - 1.0) * 250.0) / 1000.0


def _assign(types, seq_nt_desc):
    """Greedy placement. Returns list of (seq_id, type_idx, q_lo, q_len) or None."""
    cnt = {i: 8 for i in range(len(types))}
    pieces = []
    for seq_id, nt in seq_nt_desc:
        need, q_lo = nt, 0
        cands = sorted(range(len(types)), key=lambda i: (types[i][0], -types[i][1]))
        for i in cands:
            k, q = types[i]
            if k < nt:
                continue
            while cnt[i] > 0 and need > 0:
                take = min(q, need)
                pieces.append((seq_id, i, q_lo, take))
                q_lo += take
                need -= take
                cnt[i] -= 1
            if need == 0:
                break
        if need:
            return None
    return pieces


def _make_schedule(seq_lens):
    nts = [_ntiles(L) for L in seq_lens]
    seq_nt_desc = sorted(enumerate(nts), key=lambda x: -x[1])
    kvals = sorted(set(nts))
    shapes = [(k, q) for k in kvals for q in range(1, k + 1)]
    PROJ = 2.46
    FIXED = 0.8

    def cost(t):
        return t[0] * t[1] * _attn_unit(t[1]) + PROJ * (t[0] + t[1]) + FIXED

    best_cost, best = float("inf"), None
    for r in (2, 3, 4):
        for types in cwr(shapes, r):
            c = sum(cost(t) for t in types)
            if c >= best_cost:
                continue
            pieces = _assign(list(types), seq_nt_desc)
            if pieces is not None:
                best_cost, best = c, (list(types), pieces)
    types, pieces = best
    # order rect types big->small for the pipeline; instance slots per type
    order = sorted(range(len(types)), key=lambda i: (-types[i][0], -types[i][1]))
    remap = {old: new for new, old in enumerate(order)}
    rects = [types[i] for i in order]
    inst = [[] for _ in rects]  # per rect: list of (seq, q_lo, q_len)
    for seq_id, ti, q_lo, q_len in pieces:
        inst[remap[ti]].append((seq_id, q_lo, q_len))
    assert all(len(x) <= N_CORES for x in inst)
    return rects, inst


# ---------------------------------------------------------------------------
# Bass program for a given rect structure.
# ---------------------------------------------------------------------------

def _chunks(total_cols):
    out = []
    qs = 0
    while qs < total_cols:
        w = min(512, total_cols - qs)
        out.append((qs, w))
        qs += w
    return out


def _build_bass(structure: tuple):
    """structure: tuple of (kb, qb) tile-counts per rect."""
    from contextlib import ExitStack

    import concourse.bass as bass
    import concourse.mybir as mybir
    import concourse.tile as tile
    from concourse import bacc

    fp32 = mybir.dt.float32
    fp16 = mybir.dt.float16
    Exp = mybir.ActivationFunctionType.Exp
    mult = mybir.AluOpType.mult
    add = mybir.AluOpType.add

    R = len(structure)
    KB = [kb for kb, _ in structure]
    QB = [qb for _, qb in structure]

    nc = bacc.Bacc("TRN2", target_bir_lowering=False, debug=False)

    xk_d = [
        nc.dram_tensor(f"xk{r}", [P, KC, KB[r] * P], fp16, kind="ExternalInput").ap()
        for r in range(R)
    ]
    xq_d = [
        nc.dram_tensor(f"xq{r}", [P, KC, QB[r] * P], fp16, kind="ExternalInput").ap()
        for r in range(R)
    ]
    kbias_d = [
        nc.dram_tensor(f"kbias{r}", [P, KB[r]], fp32, kind="ExternalInput").ap()
        for r in range(R)
    ]
    w_d = {
        name: nc.dram_tensor(name, [P, KC, D], fp16, kind="ExternalInput").ap()
        for name in ("wq", "wk", "wv", "wo")
    }
    bo_d = nc.dram_tensor("bo", [D], fp32, kind="ExternalInput").ap()
    out_d = [
        nc.dram_tensor(f"out{r}", [QB[r] * P, D], fp16, kind="ExternalOutput").ap()
        for r in range(R)
    ]

    with ExitStack() as ctx:
        tc = ctx.enter_context(tile.TileContext(nc))
        singles = ctx.enter_context(tc.tile_pool(name="singles", bufs=1))
        big = ctx.enter_context(tc.tile_pool(name="big", bufs=1))
        epool = ctx.enter_context(tc.tile_pool(name="epool", bufs=3))
        opool = ctx.enter_context(tc.tile_pool(name="opool", bufs=4))
        mmps = ctx.enter_context(tc.tile_pool(name="mmps", bufs=2, space="PSUM"))
        scps = ctx.enter_context(tc.tile_pool(name="scps", bufs=2, space="PSUM"))
        accps = ctx.enter_context(tc.tile_pool(name="accps", bufs=1, space="PSUM"))

        # ---- constants / weights / x (host-pretransposed, feature-major) ----
        # DMA emission order: rect0's k-side + the weights it needs first, so
        # the PE pipeline starts as early as possible.
        ones64 = singles.tile([P, DH], fp16)
        nc.vector.memset(ones64, 1.0)

        # PE warmup: the first real matmul waits ~8us on input DMAs; dummy
        # matmuls on memset data keep the PE busy through that window so the
        # HAM clock gate is at full rate when real work arrives.
        warm_mov = singles.tile([P, 512], fp16)
        nc.vector.memset(warm_mov, 0.0)
        warm_ps = mmps.tile([P, 512], fp32, name="warm_ps", tag="mm")
        for _ in range(14):
            nc.tensor.matmul(
                warm_ps, warm_mov[:, 0:P], warm_mov, start=True, stop=True
            )
        w_sb = {
            name: singles.tile([P, KC, D], fp16, name=f"w_{name}")
            for name in ("wv", "wq", "wk", "wo")
        }
        xT = [big.tile([P, KC, KB[r] * P], fp16, name=f"xT{r}") for r in range(R)]
        xqT = [big.tile([P, KC, QB[r] * P], fp16, name=f"xqT{r}") for r in range(R)]
        kbias_sb = [
            singles.tile([P, KB[r]], fp32, name=f"kbias{r}") for r in range(R)
        ]

        # blocks run smallest rect first (its inputs arrive fastest), then the
        # big rects in order; the last rect should have small qb (short tail).
        border = [R - 1] + list(range(R - 1)) if R > 1 else [0]

        def dma_x(r):
            for kc in range(KC):
                nc.sync.dma_start(out=xT[r][:, kc, :], in_=xk_d[r][:, kc, :])
            nc.sync.dma_start(out=xqT[r], in_=xq_d[r])
            nc.sync.dma_start(out=kbias_sb[r], in_=kbias_d[r])

        # wv feeds the very first compute (V of the first rect): issue it first,
        # kc-split across queues so it lands fast
        for kc in range(KC):
            nc.sync.dma_start(out=w_sb["wv"][:, kc, :], in_=w_d["wv"][:, kc, :])
        dma_x(border[0])
        for name in ("wq", "wk"):
            nc.sync.dma_start(out=w_sb[name], in_=w_d[name])
        for p in range(1, len(border)):
            dma_x(border[p])
        nc.sync.dma_start(out=w_sb["wo"], in_=w_d["wo"])
        bo_rep = singles.tile([P, D], fp32)
        bo_bcast = bass.AP(tensor=bo_d.tensor, offset=bo_d.offset, ap=[[0, P], [1, D]])
        nc.gpsimd.dma_start(out=bo_rep, in_=bo_bcast)

        QT = [big.tile([P, KC, QB[r] * P], fp16, name=f"QT{r}") for r in range(R)]
        KT = [big.tile([P, KC, KB[r] * P], fp16, name=f"KT{r}") for r in range(R)]
        V = [big.tile([P, KB[r], D], fp16, name=f"V{r}") for r in range(R)]
        outT = [big.tile([P, KC, QB[r] * P], fp16, name=f"oT{r}") for r in range(R)]

        # ---- unit generators (each unit = one closure, emitted by scheduler) ----
        def v_units(r, st):
            ps_box = []

            def mk_mm(kc):
                def emit():
                    if not ps_box:
                        ps_box.append(mmps.tile([P, 512], fp32, name="v_ps", tag="mm"))
                    nc.tensor.matmul(
                        ps_box[0],
                        xT[r][:, kc, st * P : (st + 1) * P],
                        w_sb["wv"][:, kc, :],
                        start=(kc == 0),
                        stop=(kc == KC - 1),
                    )
                return emit

            def fin():
                nc.vector.tensor_copy(out=V[r][:, st, :], in_=ps_box[0])

            return [mk_mm(kc) for kc in range(KC)] + [fin]

        def qtkt_units(r, hp, dst, src, wname, qs, w):
            ps_box = []

            def mk_mm(kc):
                def emit():
                    if not ps_box:
                        ps_box.append(mmps.tile([P, 512], fp32, name="qk_ps", tag="mm"))
                    nc.tensor.matmul(
                        ps_box[0][:, :w],
                        w_sb[wname][:, kc, hp * P : (hp + 1) * P],
                        src[:, kc, qs : qs + w],
                        start=(kc == 0),
                        stop=(kc == KC - 1),
                    )
                return emit

            def fin():
                nc.vector.tensor_copy(out=dst[:, hp, qs : qs + w], in_=ps_box[0][:, :w])

            return [mk_mm(kc) for kc in range(KC)] + [fin]

        def outproj_units(r, st, split_dma=False):
            ps_box = []

            def mk_mm(hc):
                def emit():
                    if not ps_box:
                        ps_box.append(mmps.tile([P, 512], fp32, name="fo_ps", tag="mm"))
                    nc.tensor.matmul(
                        ps_box[0],
                        outT[r][:, hc, st * P : (st + 1) * P],
                        w_sb["wo"][:, hc, :],
                        start=(hc == 0),
                        stop=(hc == KC - 1),
                    )
                return emit

            def fin():
                fout = opool.tile([P, D], fp16, tag="fout")
                nc.vector.tensor_tensor(fout, ps_box[0], bo_rep, add)
                if split_dma:
                    for i in range(4):
                        nc.sync.dma_start(
                            out=out_d[r][st * P + i * 32 : st * P + (i + 1) * 32, :],
                            in_=fout[i * 32 : (i + 1) * 32, :],
                        )
                else:
                    nc.sync.dma_start(out=out_d[r][st * P : (st + 1) * P, :], in_=fout)

            return [mk_mm(hc) for hc in range(KC)] + [fin]

        def attn_chunk(r, hp, qs, w, filler, iters_left):
            o_ps = accps.tile([P, 512], fp32, name="o_ps", tag="o_ps")
            d_ps = accps.tile([P, 512], fp32, name="d_ps", tag="d_ps")
            nt = KB[r]

            def emit_scores_exp(kt):
                s_pair = scps.tile([P, 1024], fp32, name="s_pair", tag="s_pair")
                nc.tensor.matmul(
                    s_pair[:, 0:w],
                    KT[r][0:DH, hp, kt * P : (kt + 1) * P],
                    QT[r][0:DH, hp, qs : qs + w],
                    start=True, stop=True, tile_position=(0, 0),
                )
                nc.tensor.matmul(
                    s_pair[:, 512 : 512 + w],
                    KT[r][DH:P, hp, kt * P : (kt + 1) * P],
                    QT[r][DH:P, hp, qs : qs + w],
                    start=True, stop=True, tile_position=(DH, 0),
                )
                e_pair = epool.tile([P, 2, 512], fp16, name="e_pair", tag="e_pair")
                nc.scalar.activation(
                    e_pair[:, :, :w],
                    s_pair.rearrange("p (h q) -> p h q", h=2)[:, :, :w],
                    Exp, bias=kbias_sb[r][:, kt : kt + 1], scale=DH**-0.5,
                )
                return e_pair

            def emit_pv(kt, e_pair):
                first, last = kt == 0, kt == nt - 1
                nc.tensor.matmul(
                    o_ps[0:DH, :w], V[r][:, kt, hp * P : hp * P + DH],
                    e_pair[:, 0, :w], start=first, stop=last,
                    tile_position=(0, 0), skip_group_check=True,
                )
                nc.tensor.matmul(
                    o_ps[DH:P, :w], V[r][:, kt, hp * P + DH : (hp + 1) * P],
                    e_pair[:, 1, :w], start=first, stop=last,
                    tile_position=(0, DH), skip_group_check=True,
                )
                nc.tensor.matmul(
                    d_ps[0:DH, :w], ones64, e_pair[:, 0, :w],
                    start=first, stop=last,
                    tile_position=(0, 0), skip_group_check=True,
                )
                nc.tensor.matmul(
                    d_ps[DH:P, :w], ones64, e_pair[:, 1, :w],
                    start=first, stop=last,
                    tile_position=(0, DH), skip_group_check=True,
                )

            pending = None
            for kt in range(nt):
                e_pair = emit_scores_exp(kt)
                if pending is not None:
                    emit_pv(*pending)
                pending = (kt, e_pair)
                if filler and iters_left[0] > 0:
                    k = -(-len(filler) // iters_left[0])
                    for _ in range(min(k, len(filler))):
                        filler.pop(0)()
                iters_left[0] -= 1
            emit_pv(*pending)
            rrep = epool.tile([P, 512], fp32, tag="rrep", bufs=2)
            nc.vector.reciprocal_approx_fast(out=rrep[:, :w], in_=d_ps[:, :w])
            nc.vector.tensor_tensor(
                outT[r][:, hp, qs : qs + w], o_ps[:, :w], rrep[:, :w], mult
            )

        # ---- choreographed emission ----
        def all_qtkt(r, hp):
            units = []
            for qs, w in _chunks(QB[r] * P):
                units.extend(qtkt_units(r, hp, QT[r], xqT[r], "wq", qs, w))
            for qs, w in _chunks(KB[r] * P):
                units.extend(qtkt_units(r, hp, KT[r], xT[r], "wk", qs, w))
            return units

        blocks = [(r, hp) for r in border for hp in range(KC)]
        nb = len(blocks)
        during = [[] for _ in blocks]
        # QT/KT of block j+1 emitted during block j
        for j in range(1, nb):
            r, hp = blocks[j]
            during[j - 1].extend(all_qtkt(r, hp))
        # V of position-p rect emitted during position p-1's middle blocks
        for p in range(1, len(border)):
            vs = [u for st in range(KB[border[p]]) for u in v_units(border[p], st)]
            base = (p - 1) * KC
            half = (len(vs) + 1) // 2
            during[base + 1].extend(vs[:half])
            during[base + 2].extend(vs[half:])
        # O-proj of position-p rect spread across position p+1's blocks
        for p in range(len(border) - 1):
            ous = [u for st in range(QB[border[p]]) for u in outproj_units(border[p], st)]
            tgt = [(p + 1) * KC + j for j in range(KC)]
            per = -(-len(ous) // len(tgt))
            for i, j in enumerate(tgt):
                during[j].extend(ous[i * per : (i + 1) * per])

        # pre-phase: V + QT/KT(hp0) of the first block rect
        r0b = border[0]
        for st in range(KB[r0b]):
            for u in v_units(r0b, st):
                u()
        for u in all_qtkt(r0b, 0):
            u()

        last_r = border[-1]
        absorbed = 0
        filler: list = []
        for j, (r, hp) in enumerate(blocks):
            filler.extend(during[j])
            ch = _chunks(QB[r] * P)
            iters_left = [len(ch) * KB[r]]
            for ci, (qs, w) in enumerate(ch):
                if j == nb - 1 and ci == len(ch) - 1 and qs > 0:
                    # final chunk of the final block: absorb O-proj of the
                    # q-tiles completed by this block's earlier chunks
                    absorbed = qs // P
                    for st in range(absorbed):
                        filler.extend(outproj_units(r, st))
                attn_chunk(r, hp, qs, w, filler, iters_left)
            while filler:
                filler.pop(0)()

        # tail: O-proj of the final rect's remaining q-tiles (split the out
        # DMA across queues — it is the last thing on the critical path)
        for st in range(absorbed, QB[last_r]):
            for u in outproj_units(last_r, st, split_dma=True):
                u()

    nc.compile()
    return nc


def _get_program(structure: tuple):
    if structure not in _BUILD_CACHE:
        _BUILD_CACHE[structure] = _build_bass(structure)
    return _BUILD_CACHE[structure]


# ---------------------------------------------------------------------------
# Host wrapper
# ---------------------------------------------------------------------------

def kernel(x, seq_lens, Wq, Wk, Wv, Wo, bo) -> np.ndarray:
    from concourse.bass_utils import run_bass_kernel_spmd

    x = np.ascontiguousarray(np.asarray(x, dtype=np.float32))
    seq_lens_np = np.asarray(seq_lens, dtype=np.int32)
    weights = {}
    for name, w in (("wq", Wq), ("wk", Wk), ("wv", Wv), ("wo", Wo)):
        w = np.asarray(w, dtype=np.float32)
        weights[name] = np.ascontiguousarray(
            w.reshape(KC, P, D).transpose(1, 0, 2).astype(np.float16)
        )
    bo = np.ascontiguousarray(np.asarray(bo, dtype=np.float32))

    rects, inst = _make_schedule(seq_lens_np)
    structure = tuple((kb, qb) for kb, qb in rects)
    nc = _get_program(structure)

    x16 = x.astype(np.float16)
    pos = np.arange(0, S, dtype=np.int32)

    def feat_major(rows):  # [T, D] -> [P, KC, T]
        return np.ascontiguousarray(rows.T.reshape(KC, P, -1).transpose(1, 0, 2))

    in_maps = []
    for c in range(N_CORES):
        m = dict(weights)
        m["bo"] = bo
        for r, (kb, qb) in enumerate(rects):
            piece = inst[r][c] if c < len(inst[r]) else None
            if piece is None:
                m[f"xk{r}"] = np.zeros((P, KC, kb * P), dtype=np.float16)
                m[f"xq{r}"] = np.zeros((P, KC, qb * P), dtype=np.float16)
                m[f"kbias{r}"] = np.full((P, kb), -60.0, dtype=np.float32)
            else:
                seq, q_lo, q_len = piece
                L = int(seq_lens_np[seq])
                m[f"xk{r}"] = feat_major(x16[seq, : kb * P])
                xq = np.zeros((qb * P, D), dtype=np.float16)
                xq[: q_len * P] = x16[seq, q_lo * P : (q_lo + q_len) * P]
                m[f"xq{r}"] = feat_major(xq)
                kb_mask = np.where(pos[: kb * P] < L, 0.0, -60.0).astype(np.float32)
                m[f"kbias{r}"] = np.ascontiguousarray(
                    kb_mask.reshape(kb, P).T
                )
        in_maps.append(m)

    trace = bool(int(os.environ.get("KERNEL_TRACE", "0")))
    res = run_bass_kernel_spmd(
        nc, in_maps, core_ids=list(range(N_CORES)), trace=trace
    )
    kernel.last_results = res

    out = np.zeros((B, S, D), dtype=np.float32)
    for r in range(len(rects)):
        for c, piece in enumerate(inst[r]):
            if piece is None:
                continue
            seq, q_lo, q_len = piece
            L = int(seq_lens_np[seq])
            lo, hi = q_lo * P, min((q_lo + q_len) * P, S)
            out[seq, lo:hi] = res.results[c][f"out{r}"][: hi - lo].astype(np.float32)
            # zero padded positions within this piece's row range
            if L < hi:
                out[seq, max(L, lo) : hi] = 0.0
    return out


# revision 33
# speedup vs baseline: 1.0224x; 1.0224x over previous
"""Ragged-sequence multi-head attention (B=16, S=1024, D=512, H=8, DH=64)
for 8 Trainium2 NeuronCores.

Strategy: the 16 sequences (known lengths at call time) are packed into a
set of rectangle types (kb, qb) in 128-token tile units.  Each rect type
is instantiated once per core (SPMD); an instance processes one
(sequence, q-tile-range) piece: attention of qb q-tiles against the
sequence's first kb k-tiles, with per-k-position additive masking of the
exp() activation.  Long sequences split their q-range across cores
(duplicating their K/V projection), short ones ride in small rects.  A
runtime search picks the rect multiset minimizing modeled PE time.

Per-core pipeline per rect (fp16 matmul operands, fp32 accumulation):
  1. xT, xqT arrive pre-transposed (feature-major fp16, host-prepared)
  2. KT = Wk^T x^T, QT = Wq^T xq^T (feature-major), V in [tok, d] layout
  3. per head-pair, per q-chunk, per k-tile:
       scoresT[k, q] = K^T q            (row-packed head pair)
       expT = exp(scale*scoresT + kbias) (ACT engine -> fp16, masked keys -> 0)
       outT[d, q]  += V^T expT          (col-packed head pair)
       denom[., q] += 1^T expT          (col-packed, rows replicated)
  4. outT_norm = outT * reciprocal(denom)
  5. out[tok, d] = outT_norm^T Wo + bo  -> DMA out (host masks padded rows)
"""

import os
from itertools import combinations_with_replacement as cwr

import numpy as np

B, S, D = 16, 1024, 512
H, DH = 8, 64
N_CORES = 8
P = 128
KC = D // P  # 4 contraction chunks of 128

_BUILD_CACHE: dict = {}


def _ntiles(L: int) -> int:
    return max(1, (int(L) + P - 1) // P)


# ---------------------------------------------------------------------------
# Runtime scheduler: pick rect types and assign (seq, q-range) pieces.
# ---------------------------------------------------------------------------

def _attn_unit(q: int) -> float:
    w = min(q, 4) * 128
    return (1050.0 + (512.0 / w - 1.0) * 250.0) / 1000.0


def _assign(types, seq_nt_desc):
    """Greedy placement. Returns list of (seq_id, type_idx, q_lo, q_len) or None."""
    cnt = {i: 8 for i in range(len(types))}
    pieces = []
    for seq_id, nt in seq_nt_desc:
        need, q_lo = nt, 0
        cands = sorted(range(len(types)), key=lambda i: (types[i][0], -types[i][1]))
        for i in cands:
            k, q = types[i]
            if k < nt:
                continue
            while cnt[i] > 0 and need > 0:
                take = min(q, need)
                pieces.append((seq_id, i, q_lo, take))
                q_lo += take
                need -= take
                cnt[i] -= 1
            if need == 0:
                break
        if need:
            return None
    return pieces


def _make_schedule(seq_lens):
    nts = [_ntiles(L) for L in seq_lens]
    seq_nt_desc = sorted(enumerate(nts), key=lambda x: -x[1])
    kvals = sorted(set(nts))
    shapes = [(k, q) for k in kvals for q in range(1, k + 1)]
    PROJ = 2.46
    FIXED = 0.8

    def cost(t):
        return t[0] * t[1] * _attn_unit(t[1]) + PROJ * (t[0] + t[1]) + FIXED

    best_cost, best = float("inf"), None
    for r in (2, 3, 4):
        for types in cwr(shapes, r):
            c = sum(cost(t) for t in types)
            if c >= best_cost:
                continue
            pieces = _assign(list(types), seq_nt_desc)
            if pieces is not None:
                best_cost, best = c, (list(types), pieces)
    types, pieces = best
    # order rect types big->small for the pipeline; instance slots per type
    order = sorted(range(len(types)), key=lambda i: (-types[i][0], -types[i][1]))
    remap = {old: new for new, old in enumerate(order)}
    rects = [types[i] for i in order]
    inst = [[] for _ in rects]  # per rect: list of (seq, q_lo, q_len)
    for seq_id, ti, q_lo, q_len in pieces:
        inst[remap[ti]].append((seq_id, q_lo, q_len))
    assert all(len(x) <= N_CORES for x in inst)
    return rects, inst


# ---------------------------------------------------------------------------
# Bass program for a given rect structure.
# ---------------------------------------------------------------------------

def _chunks(total_cols):
    out = []
    qs = 0
    while qs < total_cols:
        w = min(512, total_cols - qs)
        out.append((qs, w))
        qs += w
    return out


def _build_bass(structure: tuple):
    """structure: tuple of (kb, qb) tile-counts per rect."""
    from contextlib import ExitStack

    import concourse.bass as bass
    import concourse.mybir as mybir
    import concourse.tile as tile
    from concourse import bacc

    fp32 = mybir.dt.float32
    fp16 = mybir.dt.float16
    Exp = mybir.ActivationFunctionType.Exp
    mult = mybir.AluOpType.mult
    add = mybir.AluOpType.add

    R = len(structure)
    KB = [kb for kb, _ in structure]
    QB = [qb for _, qb in structure]

    nc = bacc.Bacc("TRN2", target_bir_lowering=False, debug=False)

    xk_d = [
        nc.dram_tensor(f"xk{r}", [P, KC, KB[r] * P], fp16, kind="ExternalInput").ap()
        for r in range(R)
    ]
    xq_d = [
        nc.dram_tensor(f"xq{r}", [P, KC, QB[r] * P], fp16, kind="ExternalInput").ap()
        for r in range(R)
    ]
    kbias_d = [
        nc.dram_tensor(f"kbias{r}", [P, KB[r]], fp32, kind="ExternalInput").ap()
        for r in range(R)
    ]
    w_d = {
        name: nc.dram_tensor(name, [P, KC, D], fp16, kind="ExternalInput").ap()
        for name in ("wq", "wk", "wv", "wo")
    }
    bo_d = nc.dram_tensor("bo", [D], fp32, kind="ExternalInput").ap()
    out_d = [
        nc.dram_tensor(f"out{r}", [QB[r] * P, D], fp16, kind="ExternalOutput").ap()
        for r in range(R)
    ]

    with ExitStack() as ctx:
        tc = ctx.enter_context(tile.TileContext(nc))
        singles = ctx.enter_context(tc.tile_pool(name="singles", bufs=1))
        big = ctx.enter_context(tc.tile_pool(name="big", bufs=1))
        epool = ctx.enter_context(tc.tile_pool(name="epool", bufs=3))
        opool = ctx.enter_context(tc.tile_pool(name="opool", bufs=4))
        mmps = ctx.enter_context(tc.tile_pool(name="mmps", bufs=2, space="PSUM"))
        scps = ctx.enter_context(tc.tile_pool(name="scps", bufs=2, space="PSUM"))
        accps = ctx.enter_context(tc.tile_pool(name="accps", bufs=1, space="PSUM"))

        # ---- constants / weights / x (host-pretransposed, feature-major) ----
        # DMA emission order: rect0's k-side + the weights it needs first, so
        # the PE pipeline starts as early as possible.
        ones64 = singles.tile([P, DH], fp16)
        nc.vector.memset(ones64, 1.0)
        w_sb = {
            name: singles.tile([P, KC, D], fp16, name=f"w_{name}")
            for name in ("wv", "wq", "wk", "wo")
        }
        xT = [big.tile([P, KC, KB[r] * P], fp16, name=f"xT{r}") for r in range(R)]
        xqT = [big.tile([P, KC, QB[r] * P], fp16, name=f"xqT{r}") for r in range(R)]
        kbias_sb = [
            singles.tile([P, KB[r]], fp32, name=f"kbias{r}") for r in range(R)
        ]

        # blocks run smallest rect first (its inputs arrive fastest), then the
        # big rects in order; the last rect should have small qb (short tail).
        border = [R - 1] + list(range(R - 1)) if R > 1 else [0]

        def dma_x(r):
            for kc in range(KC):
                nc.sync.dma_start(out=xT[r][:, kc, :], in_=xk_d[r][:, kc, :])
            nc.sync.dma_start(out=xqT[r], in_=xq_d[r])
            nc.sync.dma_start(out=kbias_sb[r], in_=kbias_d[r])

        # wv feeds the very first compute (V of the first rect): issue it first,
        # kc-split across queues so it lands fast
        for kc in range(KC):
            nc.sync.dma_start(out=w_sb["wv"][:, kc, :], in_=w_d["wv"][:, kc, :])
        dma_x(border[0])
        for name in ("wq", "wk"):
            nc.sync.dma_start(out=w_sb[name], in_=w_d[name])
        for p in range(1, len(border)):
            dma_x(border[p])
        nc.sync.dma_start(out=w_sb["wo"], in_=w_d["wo"])
        bo_rep = singles.tile([P, D], fp32)
        bo_bcast = bass.AP(tensor=bo_d.tensor, offset=bo_d.offset, ap=[[0, P], [1, D]])
        nc.gpsimd.dma_start(out=bo_rep, in_=bo_bcast)

        QT = [big.tile([P, KC, QB[r] * P], fp16, name=f"QT{r}") for r in range(R)]
        KT = [big.tile([P, KC, KB[r] * P], fp16, name=f"KT{r}") for r in range(R)]
        V = [big.tile([P, KB[r], D], fp16, name=f"V{r}") for r in range(R)]
        outT = [big.tile([P, KC, QB[r] * P], fp16, name=f"oT{r}") for r in range(R)]

        # ---- unit generators (each unit = one closure, emitted by scheduler) ----
        def v_units(r, st):
            ps_box = []

            def mk_mm(kc):
                def emit():
                    if not ps_box:
                        ps_box.append(mmps.tile([P, 512], fp32, name="v_ps", tag="mm"))
                    nc.tensor.matmul(
                        ps_box[0],
                        xT[r][:, kc, st * P : (st + 1) * P],
                        w_sb["wv"][:, kc, :],
                        start=(kc == 0),
                        stop=(kc == KC - 1),
                    )
                return emit

            def fin():
                nc.vector.tensor_copy(out=V[r][:, st, :], in_=ps_box[0])

            return [mk_mm(kc) for kc in range(KC)] + [fin]

        def qtkt_units(r, hp, dst, src, wname, qs, w):
            ps_box = []

            def mk_mm(kc):
                def emit():
                    if not ps_box:
                        ps_box.append(mmps.tile([P, 512], fp32, name="qk_ps", tag="mm"))
                    nc.tensor.matmul(
                        ps_box[0][:, :w],
                        w_sb[wname][:, kc, hp * P : (hp + 1) * P],
                        src[:, kc, qs : qs + w],
                        start=(kc == 0),
                        stop=(kc == KC - 1),
                    )
                return emit

            def fin():
                nc.vector.tensor_copy(out=dst[:, hp, qs : qs + w], in_=ps_box[0][:, :w])

            return [mk_mm(kc) for kc in range(KC)] + [fin]

        def outproj_units(r, st, split_dma=False):
            ps_box = []

            def mk_mm(hc):
                def emit():
                    if not ps_box:
                        ps_box.append(mmps.tile([P, 512], fp32, name="fo_ps", tag="mm"))
                    nc.tensor.matmul(
                        ps_box[0],
                        outT[r][:, hc, st * P : (st + 1) * P],
                        w_sb["wo"][:, hc, :],
                        start=(hc == 0),
                        stop=(hc == KC - 1),
                    )
                return emit

            def fin():
                fout = opool.tile([P, D], fp16, tag="fout")
                nc.vector.tensor_tensor(fout, ps_box[0], bo_rep, add)
                if split_dma:
                    for i in range(4):
                        nc.sync.dma_start(
                            out=out_d[r][st * P + i * 32 : st * P + (i + 1) * 32, :],
                            in_=fout[i * 32 : (i + 1) * 32, :],
                        )
                else:
                    nc.sync.dma_start(out=out_d[r][st * P : (st + 1) * P, :], in_=fout)

            return [mk_mm(hc) for hc in range(KC)] + [fin]

        def attn_chunk(r, hp, qs, w, filler, iters_left):
            o_ps = accps.tile([P, 512], fp32, name="o_ps", tag="o_ps")
            d_ps = accps.tile([P, 512], fp32, name="d_ps", tag="d_ps")
            nt = KB[r]

            def emit_scores_exp(kt):
                s_pair = scps.tile([P, 1024], fp32, name="s_pair", tag="s_pair")
                nc.tensor.matmul(
                    s_pair[:, 0:w],
                    KT[r][0:DH, hp, kt * P : (kt + 1) * P],
                    QT[r][0:DH, hp, qs : qs + w],
                    start=True, stop=True, tile_position=(0, 0),
                )
                nc.tensor.matmul(
                    s_pair[:, 512 : 512 + w],
                    KT[r][DH:P, hp, kt * P : (kt + 1) * P],
                    QT[r][DH:P, hp, qs : qs + w],
                    start=True, stop=True, tile_position=(DH, 0),
                )
                e_pair = epool.tile([P, 2, 512], fp16, name="e_pair", tag="e_pair")
                nc.scalar.activation(
                    e_pair[:, :, :w],
                    s_pair.rearrange("p (h q) -> p h q", h=2)[:, :, :w],
                    Exp, bias=kbias_sb[r][:, kt : kt + 1], scale=DH**-0.5,
                )
                return e_pair

            def emit_pv(kt, e_pair):
                first, last = kt == 0, kt == nt - 1
                nc.tensor.matmul(
                    o_ps[0:DH, :w], V[r][:, kt, hp * P : hp * P + DH],
                    e_pair[:, 0, :w], start=first, stop=last,
                    tile_position=(0, 0), skip_group_check=True,
                )
                nc.tensor.matmul(
                    o_ps[DH:P, :w], V[r][:, kt, hp * P + DH : (hp + 1) * P],
                    e_pair[:, 1, :w], start=first, stop=last,
                    tile_position=(0, DH), skip_group_check=True,
                )
                nc.tensor.matmul(
                    d_ps[0:DH, :w], ones64, e_pair[:, 0, :w],
                    start=first, stop=last,
                    tile_position=(0, 0), skip_group_check=True,
                )
                nc.tensor.matmul(
                    d_ps[DH:P, :w], ones64, e_pair[:, 1, :w],
                    start=first, stop=last,
                    tile_position=(0, DH), skip_group_check=True,
                )

            pending = None
            for kt in range(nt):
                e_pair = emit_scores_exp(kt)
                if pending is not None:
                    emit_pv(*pending)
                pending = (kt, e_pair)
                if filler and iters_left[0] > 0:
                    k = -(-len(filler) // iters_left[0])
                    for _ in range(min(k, len(filler))):
                        filler.pop(0)()
                iters_left[0] -= 1
            emit_pv(*pending)
            rrep = epool.tile([P, 512], fp32, tag="rrep", bufs=2)
            nc.vector.reciprocal_approx_fast(out=rrep[:, :w], in_=d_ps[:, :w])
            nc.vector.tensor_tensor(
                outT[r][:, hp, qs : qs + w], o_ps[:, :w], rrep[:, :w], mult
            )

        # ---- choreographed emission ----
        def all_qtkt(r, hp):
            units = []
            for qs, w in _chunks(QB[r] * P):
                units.extend(qtkt_units(r, hp, QT[r], xqT[r], "wq", qs, w))
            for qs, w in _chunks(KB[r] * P):
                units.extend(qtkt_units(r, hp, KT[r], xT[r], "wk", qs, w))
            return units

        blocks = [(r, hp) for r in border for hp in range(KC)]
        nb = len(blocks)
        during = [[] for _ in blocks]
        # QT/KT of block j+1 emitted during block j
        for j in range(1, nb):
            r, hp = blocks[j]
            during[j - 1].extend(all_qtkt(r, hp))
        # V of position-p rect emitted during position p-1's middle blocks
        for p in range(1, len(border)):
            vs = [u for st in range(KB[border[p]]) for u in v_units(border[p], st)]
            base = (p - 1) * KC
            half = (len(vs) + 1) // 2
            during[base + 1].extend(vs[:half])
            during[base + 2].extend(vs[half:])
        # O-proj of position-p rect spread across position p+1's blocks
        for p in range(len(border) - 1):
            ous = [u for st in range(QB[border[p]]) for u in outproj_units(border[p], st)]
            tgt = [(p + 1) * KC + j for j in range(KC)]
            per = -(-len(ous) // len(tgt))
            for i, j in enumerate(tgt):
                during[j].extend(ous[i * per : (i + 1) * per])

        # pre-phase: V + QT/KT(hp0) of the first block rect
        r0b = border[0]
        for st in range(KB[r0b]):
            for u in v_units(r0b, st):
                u()
        for u in all_qtkt(r0b, 0):
            u()

        last_r = border[-1]
        absorbed = 0
        filler: list = []
        for j, (r, hp) in enumerate(blocks):
            filler.extend(during[j])
            ch = _chunks(QB[r] * P)
            iters_left = [len(ch) * KB[r]]
            for ci, (qs, w) in enumerate(ch):
                if j == nb - 1 and ci == len(ch) - 1 and qs > 0:
                    # final chunk of the final block: absorb O-proj of the
                    # q-tiles completed by this block's earlier chunks
                    absorbed = qs // P
                    for st in range(absorbed):
                        filler.extend(outproj_units(r, st))
                attn_chunk(r, hp, qs, w, filler, iters_left)
            while filler:
                filler.pop(0)()

        # tail: O-proj of the final rect's remaining q-tiles (split the out
        # DMA across queues — it is the last thing on the critical path)
        for st in range(absorbed, QB[last_r]):
            for u in outproj_units(last_r, st, split_dma=True):
                u()

    nc.compile()
    return nc


def _get_program(structure: tuple):
    if structure not in _BUILD_CACHE:
        _BUILD_CACHE[structure] = _build_bass(structure)
    return _BUILD_CACHE[structure]


# ---------------------------------------------------------------------------
# Host wrapper
# ---------------------------------------------------------------------------

def kernel(x, seq_lens, Wq, Wk, Wv, Wo, bo) -> np.ndarray:
    from concourse.bass_utils import run_bass_kernel_spmd

    x = np.ascontiguousarray(np.asarray(x, dtype=np.float32))
    seq_lens_np = np.asarray(seq_lens, dtype=np.int32)
    weights = {}
    for name, w in (("wq", Wq), ("wk", Wk), ("wv", Wv), ("wo", Wo)):
        w = np.asarray(w, dtype=np.float32)
        weights[name] = np.ascontiguousarray(
            w.reshape(KC, P, D).transpose(1, 0, 2).astype(np.float16)
        )
    bo = np.ascontiguousarray(np.asarray(bo, dtype=np.float32))

    rects, inst = _make_schedule(seq_lens_np)
    structure = tuple((kb, qb) for kb, qb in rects)
    nc = _get_program(structure)

    x16 = x.astype(np.float16)
    pos = np.arange(0, S, dtype=np.int32)

    def feat_major(rows):  # [T, D] -> [P, KC, T]
        return np.ascontiguousarray(rows.T.reshape(KC, P, -1).transpose(1, 0, 2))

    in_maps = []
    for c in range(N_CORES):
        m = dict(weights)
        m["bo"] = bo
        for r, (kb, qb) in enumerate(rects):
            piece = inst[r][c] if c < len(inst[r]) else None
            if piece is None:
                m[f"xk{r}"] = np.zeros((P, KC, kb * P), dtype=np.float16)
                m[f"xq{r}"] = np.zeros((P, KC, qb * P), dtype=np.float16)
                m[f"kbias{r}"] = np.full((P, kb), -60.0, dtype=np.float32)
            else:
                seq, q_lo, q_len = piece
                L = int(seq_lens_np[seq])
                m[f"xk{r}"] = feat_major(x16[seq, : kb * P])
                xq = np.zeros((qb * P, D), dtype=np.float16)
                xq[: q_len * P] = x16[seq, q_lo * P : (q_lo + q_len) * P]
                m[f"xq{r}"] = feat_major(xq)
                kb_mask = np.where(pos[: kb * P] < L, 0.0, -60.0).astype(np.float32)
                m[f"kbias{r}"] = np.ascontiguousarray(
                    kb_mask.reshape(kb, P).T
                )
        in_maps.append(m)

    trace = bool(int(os.environ.get("KERNEL_TRACE", "0")))
    res = run_bass_kernel_spmd(
        nc, in_maps, core_ids=list(range(N_CORES)), trace=trace
    )
    kernel.last_results = res

    out = np.zeros((B, S, D), dtype=np.float32)
    for r in range(len(rects)):
        for c, piece in enumerate(inst[r]):
            if piece is None:
                continue
            seq, q_lo, q_len = piece
            L = int(seq_lens_np[seq])
            lo, hi = q_lo * P, min((q_lo + q_len) * P, S)
            out[seq, lo:hi] = res.results[c][f"out{r}"][: hi - lo].astype(np.float32)
            # zero padded positions within this piece's row range
            if L < hi:
                out[seq, max(L, lo) : hi] = 0.0
    return out
